# revision 1
# baseline (speedup 1.0000x reference)
"""GAT (2-layer graph attention network) Bass kernel for Trainium2, 8 NeuronCores.

Sharding: row-wise over destination nodes (each core owns N/8 = 512 destination
rows of the NxN attention matrix and the full source-node feature table via two
small AllGathers). Params replicated.

Per-core layout:
  - scores in source-major layout: SBUF partitions = source node j (chunks of
    128), free dim = (head, dest i).
  - exp(leaky_relu(.)) via ScalarE (Prelu then Exp, both live in the
    exp_and_others ACT table set -> no table switches). Mask applied
    additively (-1000) before the activations; exp underflows to exactly 0.
  - attention-weighted sum: one matmul per (chunk, head) with stationary
    [g_h | 1] (ones column -> softmax denominator lands in psum row 8);
    feature-major accumulation in 8 PSUM banks (one per head).
"""

import sys

sys.path.insert(0, "/opt/trn_rl_repo")

import numpy as np

import concourse.bass as bass
import concourse.bacc as bacc
import concourse.mybir as mybir
from concourse import masks, tile
from concourse.bass_utils import run_bass_kernel_spmd

F32 = mybir.dt.float32
BF16 = mybir.dt.bfloat16
U8 = mybir.dt.uint8

N = 4096
IN = 512
HEADS = 8
HPH = 8
HID = HEADS * HPH  # 64
CLS = 16
NEG_SLOPE = 0.2
N_CORES = 8

AOP = mybir.AluOpType
AFT = mybir.ActivationFunctionType

# leaky-relu engine split (see build_nc); tuned from HW traces
BEST_LRELU_MOD = 2


def build_nc(n=N, n_cores=N_CORES, dve_lrelu_mod=0, reps=1):
    """Build the SPMD Bass program (one NEFF, runs on all cores).

    dve_lrelu_mod: 0 = leaky-relu always on ScalarE (Prelu);
                   k>0 = chunks with (c % k == 0) do leaky-relu on VectorE
                   instead, to balance ACT/DVE load.
    """
    I = n // n_cores        # destination rows per core
    C = n // 128            # source chunks of 128
    IB = I // 128           # 128-row blocks of own destinations
    NB = n // 512           # 512-col blocks of all nodes
    FC = IN // 128          # feature chunks

    nc = bacc.Bacc(
        "TRN2", target_bir_lowering=False, debug=False, num_devices=n_cores
    )

    x_own = nc.dram_tensor("x_own", [I, IN], F32, kind="ExternalInput")
    adjT_own = nc.dram_tensor("adjT_own", [n, I], U8, kind="ExternalInput")
    W1v = nc.dram_tensor("W1v", [IN, HID], F32, kind="ExternalInput")
    A1l = nc.dram_tensor("A1l", [HID, HEADS], F32, kind="ExternalInput")
    A1r = nc.dram_tensor("A1r", [HID, HEADS], F32, kind="ExternalInput")
    W2v = nc.dram_tensor("W2v", [HID, CLS], F32, kind="ExternalInput")
    a2lT = nc.dram_tensor("a2lT", [CLS, 1], F32, kind="ExternalInput")
    a2rT = nc.dram_tensor("a2rT", [CLS, 1], F32, kind="ExternalInput")
    selin = nc.dram_tensor("selin", [HEADS, HEADS, 128], F32, kind="ExternalInput")
    out_own = nc.dram_tensor("out_own", [I, CLS], F32, kind="ExternalOutput")

    with tile.TileContext(nc) as tc:
        with (
            tc.tile_pool(name="const", bufs=1) as const,
            tc.tile_pool(name="big", bufs=1) as big,
            tc.tile_pool(name="work", bufs=3) as work,
            tc.tile_pool(name="worklr", bufs=2) as worklr,
            tc.tile_pool(name="small", bufs=3) as small,
            tc.tile_pool(name="dram", bufs=1, space="DRAM") as dram,
        ):
            # ================= setup (scratch psum pool A) =================
            with tc.tile_pool(name="scrA", bufs=4, space="PSUM") as scrp:
                # ---------- constants ----------
                ident = const.tile([128, 128], F32, tag="ident")
                masks.make_identity(nc, ident[:])
                ident_bf = const.tile([128, 128], BF16, tag="ident_bf")
                nc.vector.tensor_copy(ident_bf[:], ident[:])
                sel = const.tile([HEADS, HEADS, 128], F32, tag="sel")
                nc.sync.dma_start(sel[:], selin.ap())
                ones_row = const.tile([1, 128], F32, tag="ones_row")
                nc.vector.memset(ones_row[:], 1.0)

                w1sb = const.tile([128, FC, HID], F32, tag="w1sb")
                nc.sync.dma_start(
                    w1sb[:], W1v.ap().rearrange("(c p) d -> p c d", p=128)
                )
                a1l_sb = const.tile([HID, HEADS], F32, tag="a1l_sb")
                nc.sync.dma_start(a1l_sb[:], A1l.ap())
                a1r_sb = const.tile([HID, HEADS], F32, tag="a1r_sb")
                nc.sync.dma_start(a1r_sb[:], A1r.ap())
                w2sb = const.tile([HID, CLS], F32, tag="w2sb")
                nc.sync.dma_start(w2sb[:], W2v.ap())
                a2l_sb = const.tile([CLS, 1], F32, tag="a2l_sb")
                nc.sync.dma_start(a2l_sb[:], a2lT.ap())
                a2r_sb = const.tile([CLS, 1], F32, tag="a2r_sb")
                nc.sync.dma_start(a2r_sb[:], a2rT.ap())
                a1r_bf = const.tile([HID, HEADS], BF16, tag="a1r_bf")
                nc.vector.tensor_copy(a1r_bf[:], a1r_sb[:])
                w2bf = const.tile([HID, CLS], BF16, tag="w2bf")
                nc.vector.tensor_copy(w2bf[:], w2sb[:])
                a2r_bf = const.tile([CLS, 1], BF16, tag="a2r_bf")
                nc.vector.tensor_copy(a2r_bf[:], a2r_sb[:])

                # ---------- mask: mneg[j, c, i] = 0 (edge) / -1000 ----------
                mneg = big.tile([128, C, I], BF16, tag="mneg")
                for c in range(C):
                    adj_t = work.tile([128, I], U8, tag="adjt")
                    nc.sync.dma_start(
                        adj_t[:], adjT_own[c * 128:(c + 1) * 128, :]
                    )
                    nc.vector.tensor_scalar(
                        mneg[:, c, :], adj_t[:], 1000.0, -1000.0,
                        AOP.mult, AOP.add,
                    )

                # ---------- x^T, projection g1T = W1^T x^T (feature-major) --
                xsb = big.tile([128, IB, IN], F32, tag="xsb")
                nc.sync.dma_start(
                    xsb[:], x_own.ap().rearrange("(b p) f -> p b f", p=128)
                )
                g1ps = scrp.tile([HID, I], F32, tag="scr")
                for fc in range(FC):
                    xT_t = work.tile([128, I], F32, tag="xTt")
                    for ib in range(IB):
                        tp = scrp.tile([128, 128], F32, tag="scr")
                        nc.tensor.transpose(
                            tp[:], xsb[:, ib, fc * 128:(fc + 1) * 128], ident[:]
                        )
                        nc.vector.tensor_copy(
                            xT_t[:, ib * 128:(ib + 1) * 128], tp[:]
                        )
                    nc.tensor.matmul(
                        g1ps[:], w1sb[:, fc, :], xT_t[:],
                        start=(fc == 0), stop=(fc == FC - 1),
                    )
                g1T_own = const.tile([HID, I], F32, tag="g1T_own")
                nc.vector.tensor_copy(g1T_own[:], g1ps[:])
                g1T_own_bf = const.tile([HID, I], BF16, tag="g1T_own_bf")
                nc.vector.tensor_copy(g1T_own_bf[:], g1T_own[:])

                # ---------- AllGather g1 ----------
                g1_in = dram.tile([HID, I], BF16, tag="g1_in")
                g1_out = dram.tile([n_cores * HID, I], BF16, tag="g1_out")
                nc.sync.dma_start(g1_in[:], g1T_own_bf[:])
                if n_cores > 1:
                    nc.gpsimd.collective_compute(
                        "AllGather",
                        AOP.bypass,
                        replica_groups=[list(range(n_cores))],
                        ins=[g1_in[:].opt()],
                        outs=[g1_out[:].opt()],
                    )
                else:
                    nc.sync.dma_start(g1_out[:], g1_in[:])
                g1T_full = big.tile([HID, n], BF16, tag="g1T_full")
                nc.sync.dma_start(
                    g1T_full[:].rearrange("d (r i) -> d r i", r=n_cores),
                    g1_out[:].rearrange("(r d) i -> d r i", d=HID),
                )

                # ---------- node-major [g | 1] for the numerator matmuls ----
                g_sb = big.tile([128, C, HEADS, HPH + 1], BF16, tag="g_sb")
                nc.vector.memset(g_sb[:], 1.0)
                for c in range(C):
                    tp = scrp.tile([128, 128], BF16, tag="scrb")
                    nc.tensor.transpose(
                        tp[:, 0:HID], g1T_full[:, c * 128:(c + 1) * 128],
                        ident_bf[0:HID, 0:HID],
                    )
                    nc.scalar.activation(
                        g_sb[:, c, :, 0:HPH],
                        tp[:, 0:HID].rearrange("p (h d) -> p h d", h=HEADS),
                        AFT.Copy,
                    )

                # ---------- scores: sl (own rows), sr (all nodes) ----------
                slps = scrp.tile([HEADS, I], F32, tag="scr")
                nc.tensor.matmul(slps[:], a1l_sb[:], g1T_own[:], start=True, stop=True)
                sl_sb = small.tile([HEADS, I], F32, tag="sl_sb")
                nc.vector.tensor_copy(sl_sb[:], slps[:])

                slb = const.tile([128, HEADS, I], BF16, tag="slb")
                for h in range(HEADS):
                    bp = scrp.tile([128, I], F32, tag="scr")
                    nc.tensor.matmul(
                        bp[:], sel[:, h, :], sl_sb[:], start=True, stop=True
                    )
                    nc.vector.tensor_copy(slb[:, h, :], bp[:])

                sr_col = const.tile([128, C, HEADS], F32, tag="sr_col")
                for c in range(C):
                    sp = scrp.tile([128, HEADS], F32, tag="scr")
                    nc.tensor.matmul(
                        sp[:], g1T_full[:, c * 128:(c + 1) * 128], a1r_bf[:],
                        start=True, stop=True,
                    )
                    nc.vector.tensor_copy(sr_col[:, c, :], sp[:])

            # ================= layer-1 attention (8 accum banks) ===========
            num1 = const.tile([HPH + 1, HEADS, I], F32, tag="num1")
            with tc.tile_pool(name="acc1", bufs=8, space="PSUM") as accp:
                ps1 = [accp.tile([HPH + 1, I], F32, tag="acc", name=f"ps1_{h}")
                       for h in range(HEADS)]
                for _rep in range(reps):
                  for c in range(C):
                    t1 = work.tile([128, HEADS, I], BF16, tag="t1")
                    nc.vector.tensor_tensor(
                        t1[:], slb[:],
                        mneg[:, c:c + 1, :].broadcast_to([128, HEADS, I]),
                        AOP.add,
                    )
                    for h in range(HEADS):
                        nc.vector.tensor_scalar(
                            t1[:, h, :], t1[:, h, :], sr_col[:, c, h:h + 1], None,
                            AOP.add,
                        )
                    p1 = work.tile([128, HEADS, I], BF16, tag="p1")
                    if dve_lrelu_mod and (c % dve_lrelu_mod == 1 % dve_lrelu_mod):
                        w1t = worklr.tile([128, HEADS, I], BF16, tag="w1t")
                        nc.vector.tensor_scalar(
                            w1t[:], t1[:], NEG_SLOPE, None, AOP.mult
                        )
                        nc.vector.tensor_tensor(t1[:], t1[:], w1t[:], AOP.max)
                    else:
                        nc.scalar.activation(t1[:], t1[:], AFT.Prelu, alpha=NEG_SLOPE)
                    nc.scalar.activation(p1[:], t1[:], AFT.Exp)
                    for h in range(HEADS):
                        nc.tensor.matmul(
                            ps1[h][:],
                            g_sb[:, c, h, :],
                            p1[:, h, :],
                            start=(c == 0), stop=(c == C - 1),
                        )
                for h in range(HEADS):
                    if h % 2 == 0:
                        nc.scalar.activation(num1[:, h, :], ps1[h][:], AFT.Copy)
                    else:
                        nc.vector.tensor_copy(num1[:, h, :], ps1[h][:])

            # ================= epilogue + layer 2 (scratch pool B) =========
            with tc.tile_pool(name="scrB", bufs=3, space="PSUM") as scrp:
                # normalize + ELU in node-major layout
                h1T = const.tile([HID, I], BF16, tag="h1T")
                for ib in range(IB):
                    nm = scrp.tile([128, 128], F32, tag="scr")
                    for h in range(HEADS):
                        nc.tensor.transpose(
                            nm[:, h * (HPH + 1):(h + 1) * (HPH + 1)],
                            num1[:, h, ib * 128:(ib + 1) * 128],
                            ident[0:HPH + 1, 0:HPH + 1],
                        )
                    nmv = nm[:, 0:HEADS * (HPH + 1)].rearrange(
                        "p (h e) -> p h e", h=HEADS
                    )
                    rz = small.tile([128, HEADS, 1], F32, tag="rz")
                    nc.vector.reciprocal(rz[:], nmv[:, :, HPH:HPH + 1])
                    h1 = small.tile([128, HID], F32, tag="h1")
                    nc.vector.tensor_tensor(
                        h1[:].rearrange("p (h d) -> p h d", h=HEADS),
                        nmv[:, :, 0:HPH],
                        rz[:].broadcast_to([128, HEADS, HPH]),
                        AOP.mult,
                    )
                    # ELU(x) = max(x,0) + min(exp(min(x,0)) - 1, 0)
                    eneg = small.tile([128, HID], F32, tag="eneg")
                    nc.vector.tensor_scalar(eneg[:], h1[:], 0.0, None, AOP.min)
                    nc.scalar.activation(eneg[:], eneg[:], AFT.Exp)
                    nc.vector.tensor_scalar(
                        eneg[:], eneg[:], -1.0, 0.0, AOP.add, AOP.min
                    )
                    nc.vector.tensor_scalar(h1[:], h1[:], 0.0, None, AOP.max)
                    nc.vector.tensor_tensor(h1[:], h1[:], eneg[:], AOP.add)
                    tp = scrp.tile([128, 128], F32, tag="scr")
                    nc.tensor.transpose(tp[0:HID, :], h1[:], ident[:])
                    nc.vector.tensor_copy(
                        h1T[:, ib * 128:(ib + 1) * 128], tp[0:HID, :]
                    )

                # ---------- AllGather h1 ----------
                h1_in = dram.tile([HID, I], BF16, tag="h1_in")
                h1_out = dram.tile([n_cores * HID, I], BF16, tag="h1_out")
                nc.sync.dma_start(h1_in[:], h1T[:])
                if n_cores > 1:
                    nc.gpsimd.collective_compute(
                        "AllGather",
                        AOP.bypass,
                        replica_groups=[list(range(n_cores))],
                        ins=[h1_in[:].opt()],
                        outs=[h1_out[:].opt()],
                    )
                else:
                    nc.sync.dma_start(h1_out[:], h1_in[:])
                h1T_full = big.tile([HID, n], BF16, tag="h1T_full")
                nc.sync.dma_start(
                    h1T_full[:].rearrange("d (r i) -> d r i", r=n_cores),
                    h1_out[:].rearrange("(r d) i -> d r i", d=HID),
                )

                # ---------- layer-2 projection + scores ----------
                g2T_full = big.tile([CLS, n], BF16, tag="g2T_full")
                for b in range(NB):
                    gp = scrp.tile([CLS, 512], F32, tag="scr")
                    nc.tensor.matmul(
                        gp[:], w2bf[:], h1T_full[:, b * 512:(b + 1) * 512],
                        start=True, stop=True,
                    )
                    nc.vector.tensor_copy(g2T_full[:, b * 512:(b + 1) * 512], gp[:])

                g2o_ps = scrp.tile([CLS, I], F32, tag="scr")
                nc.tensor.matmul(g2o_ps[:], w2bf[:], h1T[:], start=True, stop=True)
                g2T_own = small.tile([CLS, I], F32, tag="g2T_own")
                nc.vector.tensor_copy(g2T_own[:], g2o_ps[:])

                sl2ps = scrp.tile([1, I], F32, tag="scr")
                nc.tensor.matmul(sl2ps[:], a2l_sb[:], g2T_own[:], start=True, stop=True)
                sl2_sb = small.tile([1, I], F32, tag="sl2_sb")
                nc.vector.tensor_copy(sl2_sb[:], sl2ps[:])
                sl2b_ps = scrp.tile([128, I], F32, tag="scr")
                nc.tensor.matmul(
                    sl2b_ps[:], ones_row[:], sl2_sb[:], start=True, stop=True
                )
                sl2b = const.tile([128, I], BF16, tag="sl2b")
                nc.vector.tensor_copy(sl2b[:], sl2b_ps[:])

                sr2_col = const.tile([128, C], F32, tag="sr2_col")
                for c in range(C):
                    sp = scrp.tile([128, 1], F32, tag="scr")
                    nc.tensor.matmul(
                        sp[:], g2T_full[:, c * 128:(c + 1) * 128], a2r_bf[:],
                        start=True, stop=True,
                    )
                    nc.vector.tensor_copy(sr2_col[:, c:c + 1], sp[:])

                g2nm = big.tile([128, C, CLS + 1], BF16, tag="g2nm")
                nc.vector.memset(g2nm[:], 1.0)
                for c in range(C):
                    tp = scrp.tile([128, 128], BF16, tag="scrb")
                    nc.tensor.transpose(
                        tp[:, 0:CLS], g2T_full[:, c * 128:(c + 1) * 128],
                        ident_bf[0:CLS, 0:CLS],
                    )
                    nc.vector.tensor_copy(g2nm[:, c, 0:CLS], tp[:, 0:CLS])

                # ---------- layer-2 attention ----------
                num2 = const.tile([CLS + 1, I], F32, tag="num2")
                with tc.tile_pool(name="acc2", bufs=1, space="PSUM") as accp2:
                    ps2 = accp2.tile([CLS + 1, I], F32, tag="acc2t")
                    for _rep in range(reps):
                      for c in range(C):
                        t2 = work.tile([128, I], BF16, tag="t2")
                        nc.vector.tensor_tensor(
                            t2[:], sl2b[:], mneg[:, c, :], AOP.add
                        )
                        nc.vector.tensor_scalar(
                            t2[:], t2[:], sr2_col[:, c:c + 1], None, AOP.add
                        )
                        p2 = work.tile([128, I], BF16, tag="p2")
                        if dve_lrelu_mod and (c % dve_lrelu_mod == 1 % dve_lrelu_mod):
                            w2t = worklr.tile([128, I], BF16, tag="w2t")
                            nc.vector.tensor_scalar(
                                w2t[:], t2[:], NEG_SLOPE, None, AOP.mult
                            )
                            nc.vector.tensor_tensor(t2[:], t2[:], w2t[:], AOP.max)
                        else:
                            nc.scalar.activation(
                                t2[:], t2[:], AFT.Prelu, alpha=NEG_SLOPE
                            )
                        nc.scalar.activation(p2[:], t2[:], AFT.Exp)
                        nc.tensor.matmul(
                            ps2[:],
                            g2nm[:, c, :],
                            p2[:],
                            start=(c == 0), stop=(c == C - 1),
                        )
                    nc.scalar.activation(num2[:], ps2[:], AFT.Copy)

                # ---------- layer-2 epilogue ----------
                for ib in range(IB):
                    tp2 = scrp.tile([128, 128], F32, tag="scr")
                    nc.tensor.transpose(
                        tp2[:, 0:CLS + 1],
                        num2[:, ib * 128:(ib + 1) * 128],
                        ident[0:CLS + 1, 0:CLS + 1],
                    )
                    rz2 = small.tile([128, 1], F32, tag="rz2")
                    nc.vector.reciprocal(rz2[:], tp2[:, CLS:CLS + 1])
                    o = small.tile([128, CLS], F32, tag="o")
                    nc.vector.tensor_tensor(
                        o[:], tp2[:, 0:CLS],
                        rz2[:].broadcast_to([128, CLS]),
                        AOP.mult,
                    )
                    nc.sync.dma_start(out_own[ib * 128:(ib + 1) * 128, :], o[:])

    nc.compile()
    return nc


def make_in_maps(x, adj, W1, a1l, a1r, W2, a2l, a2r, n=N, n_cores=N_CORES):
    """Host-side sharding: slice rows per core; transpose adj once (layout only)."""
    I = n // n_cores
    adjT = np.ascontiguousarray(adj.T).view(np.uint8)
    W1v = np.ascontiguousarray(np.asarray(W1, dtype=np.float32).reshape(IN, HID))
    A1l = np.zeros((HID, HEADS), dtype=np.float32)
    A1l[np.arange(HID), np.arange(HID) // HPH] = np.asarray(
        a1l, dtype=np.float32
    ).reshape(-1)
    A1r = np.zeros((HID, HEADS), dtype=np.float32)
    A1r[np.arange(HID), np.arange(HID) // HPH] = np.asarray(
        a1r, dtype=np.float32
    ).reshape(-1)
    W2v = np.ascontiguousarray(np.asarray(W2, dtype=np.float32).reshape(HID, CLS))
    a2lT = np.ascontiguousarray(np.asarray(a2l, dtype=np.float32).reshape(1, CLS).T)
    a2rT = np.ascontiguousarray(np.asarray(a2r, dtype=np.float32).reshape(1, CLS).T)
    SELIN = np.ascontiguousarray(
        np.repeat(np.eye(HEADS, dtype=np.float32)[:, :, None], 128, axis=2)
    )
    x = np.asarray(x, dtype=np.float32)
    in_maps = []
    for r in range(n_cores):
        in_maps.append({
            "x_own": np.ascontiguousarray(x[r * I:(r + 1) * I]),
            "adjT_own": np.ascontiguousarray(adjT[:, r * I:(r + 1) * I]),
            "W1v": W1v,
            "A1l": A1l,
            "A1r": A1r,
            "W2v": W2v,
            "a2lT": a2lT,
            "a2rT": a2rT,
            "selin": SELIN,
        })
    return in_maps


_NC_CACHE = {}


def kernel(x, adj, W1, a1l, a1r, W2, a2l, a2r):
    key = "full"
    if key not in _NC_CACHE:
        _NC_CACHE[key] = build_nc(dve_lrelu_mod=BEST_LRELU_MOD)
    nc = _NC_CACHE[key]
    in_maps = make_in_maps(x, adj, W1, a1l, a1r, W2, a2l, a2r)
    res = run_bass_kernel_spmd(nc, in_maps, core_ids=list(range(N_CORES)))
    out = np.concatenate([r["out_own"] for r in res.results], axis=0)
    return out.astype(np.float32)


if __name__ == "__main__":
    import reference

    inputs = reference.setup_inputs()
    inputs = {k: np.asarray(v) for k, v in inputs.items()}
    expected = np.asarray(reference.reference(**inputs))
    actual = kernel(**inputs)
    err = np.abs(actual - expected).max() / (np.abs(expected).max() + 1e-30)
    print("Relative error:", err)

